# revision 47
# baseline (speedup 1.0000x reference)
"""Trainium2 Bass kernel for BiencoderRanker span pooling + gold matching.

Data-parallel over batch B=8 across 8 NeuronCores. Per core:
  - emb[m, :] = mean(bert[s_m : e_m+1, :]) computed as a banded matmul
    A_tile.T @ bert_rows on the PE. A is a 0/1 span mask (shipped as bf16,
    DMA-upcast to float32r); the 1/len scaling is applied during PSUM
    eviction as a per-partition scalar multiply.
  - gold id matching via f32 compare/select ops on the vector engine.
scores/bounds outputs are identity reshapes of the inputs (done host-side).
"""

import os
import numpy as np

import concourse.bass as bass
import concourse.tile as tile
from concourse import bacc, mybir
from concourse.bass_utils import run_bass_kernel_spmd

B, S, H, M, G = 8, 512, 768, 5075, 3
NCORES = 8
NT = (M + 127) // 128          # 40 span tiles of 128
MP = NT * 128                  # 5120 padded span count
NF = 40                        # free-dim size for the f-major [128, NF] gold layout
NAG = 4                        # A is loaded in NAG column groups for early start
# bert is staged in SBUF as 8 overlapping row-chunks: chunk ci holds rows
# [64ci, min(64ci+128, S)). Even chunks come from HBM; odd chunks (and the
# 64-row chunk 7) are assembled from the even ones by SBUF->SBUF copies.
# Piece starts are aligned to the 64-grid so every matmul operand starts at
# partition 0. Keeping K <= ~85 also avoids a PE pipeline stall seen at K=128.
NCHUNK = 8

LAST_EXEC_NS = None            # set when KERNEL_TRACE=1

_prog_cache = {}


def _plan_pieces(smin, emax):
    """Per span-tile, decompose the token window [smin, emax] into pieces that
    each live inside one SBUF bert chunk, starting at partition 0.
    Returns list (per tile) of lists of (ci, nrows, kbase)."""
    plans = []
    for t in range(NT):
        lo, hi = int(smin[t]), int(emax[t])
        pieces = []
        pos = (lo // 64) * 64
        while pos <= hi:
            ci = pos // 64
            room = 128 if ci < NCHUNK - 1 else S - 64 * ci
            n = min(hi + 1 - pos, room)
            pieces.append((ci, n, pos))
            pos += n
        plans.append(pieces)
    return plans


def _build_program(plans):
    """Build + compile the SPMD Bass program for a given piece plan."""
    PT = sum(len(p) for p in plans)
    nc = bacc.Bacc("TRN2", target_bir_lowering=False, debug=False)
    f32, i32 = mybir.dt.float32, mybir.dt.int32
    f32r, bf16 = mybir.dt.float32r, mybir.dt.bfloat16

    # piece index ranges per A group: a small first group (so tile 0's
    # stationary operand lands fast) then ~12-piece groups.
    abounds = [0, min(4, PT)]
    while abounds[-1] < PT:
        abounds.append(min(abounds[-1] + 12, PT))
    nag = len(abounds) - 1

    bert_in = nc.dram_tensor("bert", [S, H], f32r, kind="ExternalInput")
    a_in = nc.dram_tensor("amat", [128, PT * 128], f32r, kind="ExternalInput")
    inv_in = nc.dram_tensor("inv", [128, NT], f32, kind="ExternalInput")
    se_in = nc.dram_tensor("se", [128, 2 * NF], f32, kind="ExternalInput")
    gold_in = nc.dram_tensor("gold", [128, 3 * G], f32, kind="ExternalInput")
    emb_out = nc.dram_tensor("emb", [MP, H], f32, kind="ExternalOutput")
    gid_out = nc.dram_tensor("gid", [128, NF], i32, kind="ExternalOutput")

    with tile.TileContext(nc) as tc:
        with (
            tc.tile_pool(name="const", bufs=1) as const,
            tc.tile_pool(name="psum", bufs=4, space="PSUM") as psum,
            tc.tile_pool(name="outp", bufs=10) as outp,
            tc.tile_pool(name="gtmp", bufs=2) as gtmp,
        ):
            # A group 0 first so tile 0's matmuls can start immediately.
            a_sb = [None] * nag

            def load_a(g, eng):
                lo, hi = abounds[g], abounds[g + 1]
                at = const.tile([128, (hi - lo) * 128], f32r, tag=f"amat{g}")
                eng.dma_start(at[:], a_in[:, lo * 128:hi * 128])
                a_sb[g] = at

            # Queue plan: scalar issues nothing before its evictions (so the
            # ACT table load + first eviction happen ASAP); sync interleaves
            # bert even-chunks with the A groups; gpsimd does inv + the
            # odd-chunk copies + small tensors.
            inv_sb = const.tile([128, NT], f32, tag="inv")
            nc.gpsimd.dma_start(inv_sb[:], inv_in[:])

            bert_sb = [None] * NCHUNK

            def load_chunk(ci):
                bt = const.tile([128, H], f32r, tag=f"bert{ci}")
                nc.sync.dma_start(bt[:], bert_in[64 * ci:64 * ci + 128, :])
                bert_sb[ci] = bt

            # Ordered by when the consumer tiles run: tile t uses chunk
            # ~t/5 and A group covering piece t.
            load_a(0, nc.scalar)   # scalar queue is idle until evictions
            load_chunk(0)
            if nag > 1:
                load_a(1, nc.sync)
            for ci in range(1, NCHUNK, 2):
                bt = const.tile([128, H], f32r, tag=f"bert{ci}")
                nr = min(128, S - 64 * ci)
                nc.scalar.dma_start(bt[0:nr, :], bert_in[64 * ci:64 * ci + nr, :])
                bert_sb[ci] = bt
            load_chunk(2)
            if nag > 2:
                load_a(2, nc.sync)
            load_chunk(4)
            for g in range(3, nag):
                load_a(g, nc.sync)
            load_chunk(6)

            se_sb = const.tile([128, 2 * NF], f32, tag="se")
            nc.gpsimd.dma_start(se_sb[:], se_in[:])
            gold_sb = const.tile([128, 3 * G], f32, tag="gold")
            nc.gpsimd.dma_start(gold_sb[:], gold_in[:])

            pidx = 0
            for t in range(NT):
                ot = outp.tile([128, H], f32, tag="ot")
                ps = psum.tile([128, H], f32, tag="ps")
                np_ = len(plans[t])
                for j, (ci, k, _) in enumerate(plans[t]):
                    g = next(i for i in range(nag)
                             if abounds[i] <= pidx < abounds[i + 1])
                    off = pidx - abounds[g]
                    lhsT = a_sb[g][0:k, off * 128:(off + 1) * 128]
                    rhs = bert_sb[ci]
                    first, last = (j == 0), (j == np_ - 1)
                    nc.tensor.matmul(ps[:, 0:512], lhsT, rhs[0:k, 0:512],
                                     start=first, stop=last)
                    nc.tensor.matmul(ps[:, 512:H], lhsT, rhs[0:k, 512:H],
                                     start=first, stop=last)
                    pidx += 1
                iv = inv_sb[:, t:t + 1]
                if t % 2 == 0:
                    nc.vector.tensor_scalar_mul(ot[:], ps[:], iv)
                else:
                    nc.scalar.mul(ot[:], ps[:], iv)
                eng = nc.sync if t % 2 == 0 else nc.scalar
                eng.dma_start(emb_out[128 * t:128 * (t + 1), :], ot[:])

            # gold matching in f32 (positions <=512, ids <1e5 are f32-exact):
            # acc = acc*(1-m) + id*m per gold; later golds overwrite earlier.
            starts = se_sb[:, 0:NF]
            ends = se_sb[:, NF:2 * NF]
            acc = gtmp.tile([128, NF], f32, tag="acc")
            nc.gpsimd.memset(acc[:], -1.0)
            for g in range(G):
                m1 = gtmp.tile([128, NF], f32, tag="m1")
                m2 = gtmp.tile([128, NF], f32, tag="m2")
                nc.gpsimd.tensor_scalar(m1[:], starts, gold_sb[:, g:g + 1],
                                        None, mybir.AluOpType.is_equal)
                nc.gpsimd.tensor_scalar(m2[:], ends, gold_sb[:, G + g:G + g + 1],
                                        None, mybir.AluOpType.is_equal)
                nc.gpsimd.tensor_tensor(m1[:], m1[:], m2[:],
                                        mybir.AluOpType.mult)
                # tmp = m * id ; nm = 1 - m ; acc = acc*nm + tmp
                x1 = gtmp.tile([128, NF], f32, tag="x1")
                nc.gpsimd.tensor_scalar(x1[:], m1[:],
                                        gold_sb[:, 2 * G + g:2 * G + g + 1],
                                        None, mybir.AluOpType.mult)
                nc.gpsimd.tensor_scalar(m1[:], m1[:], -1.0, 1.0,
                                        mybir.AluOpType.mult,
                                        mybir.AluOpType.add)
                nc.gpsimd.tensor_tensor(acc[:], acc[:], m1[:],
                                        mybir.AluOpType.mult)
                nc.gpsimd.tensor_tensor(acc[:], acc[:], x1[:],
                                        mybir.AluOpType.add)
            acc_i = gtmp.tile([128, NF], i32, tag="acci")
            nc.gpsimd.tensor_copy(acc_i[:], acc[:])
            nc.sync.dma_start(gid_out[:], acc_i[:])

    nc.compile()
    return nc


def _host_prep(bert_output, mention_bounds, gold_mention_bounds,
               gold_mention_bounds_mask, gold_entity_local_id):
    starts = mention_bounds[..., 0].astype(np.int64)   # [B, M]
    ends = mention_bounds[..., 1].astype(np.int64)     # [B, M] inclusive

    # union window across cores per span tile (keeps the program SPMD-uniform)
    s_pad = np.full((B, MP), 0, np.int64)
    e_pad = np.full((B, MP), -1, np.int64)
    s_pad[:, :M] = starts
    e_pad[:, :M] = ends
    st = s_pad.reshape(B, NT, 128)
    et = e_pad.reshape(B, NT, 128)
    # padded columns have s=0,e=-1 (empty span); ignore them for the window
    smin = np.where(et >= st, st, np.iinfo(np.int64).max).min(axis=(0, 2))
    emax = np.where(et >= st, et, -1).max(axis=(0, 2))
    smin = np.minimum(smin, emax)          # guard (never hit with real spans)
    smin = np.clip(smin, 0, S - 1)
    emax = np.clip(emax, 0, S - 1)
    plans = _plan_pieces(smin, emax)
    PT = sum(len(p) for p in plans)

    inv_len = (np.float32(1.0) / (ends - starts + 1).astype(np.float32))  # [B, M]
    inv_pad = np.zeros((B, MP), np.float32)
    inv_pad[:, :M] = inv_len
    inv = np.ascontiguousarray(inv_pad.reshape(B, NT, 128).transpose(0, 2, 1))

    # A mask matrices per core: [128 rows(k), PT, 128 cols(span j)] in bf16
    amats = np.zeros((B, 128, PT, 128), np.float16).astype(np.dtype("bfloat16")
        ) if False else np.zeros((B, 128, PT, 128), np.float32)
    pidx = 0
    for t in range(NT):
        s_t, e_t = st[:, t], et[:, t]                            # [B, 128]
        for (ci, k, kbase) in plans[t]:
            kk = np.arange(kbase, kbase + k)[None, :, None]      # [1, k, 1]
            msk = (kk >= s_t[:, None, :]) & (kk <= e_t[:, None, :])
            amats[:, :k, pidx, :] = msk
            pidx += 1
    amats = amats.reshape(B, 128, PT * 128)

    # starts/ends in f-major [128, NF] layout (m = p + 128 f), pad sentinel -7
    def fmaj(x, pad):
        xp = np.full((B, MP), pad, np.float32)
        xp[:, :M] = x
        return xp.reshape(B, NF, 128).transpose(0, 2, 1)  # [B, 128, NF]

    se = np.concatenate([fmaj(starts, -7), fmaj(ends, -7)], axis=2)  # [B,128,2NF]
    se = np.ascontiguousarray(se, np.float32)

    # gold triples (start, end_incl, id) with sentinels for masked/invalid
    gs = np.full((B, G), -99999, np.int64)
    ge = np.full((B, G), -99999, np.int64)
    gi = np.zeros((B, G), np.int64)
    gm = gold_mention_bounds_mask.astype(bool)
    g0 = gold_mention_bounds[..., 0].astype(np.int64)
    g1 = gold_mention_bounds[..., 1].astype(np.int64) - 1   # inclusive end
    valid = gm & (g0 >= 0)
    gs = np.where(valid, g0, gs)
    ge = np.where(valid, g1, ge)
    gi = np.where(valid, gold_entity_local_id.astype(np.int64), gi)
    gold = np.concatenate([gs, ge, gi], axis=1).astype(np.float32)  # [B, 3G]
    gold = np.ascontiguousarray(np.broadcast_to(gold[:, None, :], (B, 128, 3 * G)))

    plan_key = tuple(tuple(p) for p in plans)
    return plans, plan_key, amats, inv, se, gold


def kernel(bert_output, mention_scores, mention_bounds, gold_mention_bounds,
           gold_mention_bounds_mask, gold_entity_local_id):
    global LAST_EXEC_NS
    assert bert_output.shape == (B, S, H)
    assert mention_bounds.shape == (B, M, 2)

    plans, plan_key, amats, inv, se, gold = _host_prep(
        bert_output, mention_bounds, gold_mention_bounds,
        gold_mention_bounds_mask, gold_entity_local_id)

    if plan_key not in _prog_cache:
        _prog_cache[plan_key] = _build_program(plans)
    nc = _prog_cache[plan_key]

    bert = np.ascontiguousarray(bert_output, dtype=np.float32)
    in_maps = [
        {"bert": bert[c], "amat": amats[c], "inv": inv[c],
         "se": se[c], "gold": gold[c]}
        for c in range(NCORES)
    ]

    trace = os.environ.get("KERNEL_TRACE", "") == "1"
    if trace:
        try:
            import sys, types
            from trn_agent_boot.trn_boot import _ntff_profile_via_ctypes
            hook = _ntff_profile_via_ctypes("/opt/axon/libaxon_pjrt.so")
            mod = types.ModuleType("antenv.axon_hooks")
            mod.get_axon_ntff_profile_hook = lambda: hook
            sys.modules["antenv.axon_hooks"] = mod
        except Exception:
            trace = False

    res = run_bass_kernel_spmd(nc, in_maps, list(range(NCORES)), trace=trace)
    LAST_EXEC_NS = res.exec_time_ns

    embs = np.concatenate([res.results[c]["emb"][:M] for c in range(NCORES)], 0)
    gids = np.concatenate(
        [res.results[c]["gid"].T.ravel()[:M] for c in range(NCORES)], 0)

    return (
        embs.astype(np.float32, copy=False),
        np.asarray(mention_scores, np.float32).reshape(-1, 1),
        np.asarray(mention_bounds, np.int32).reshape(-1, 2),
        gids.astype(np.int32, copy=False).reshape(-1, 1),
    )


# revision 49
# speedup vs baseline: 1.0680x; 1.0680x over previous
"""Trainium2 Bass kernel for BiencoderRanker span pooling + gold matching.

Data-parallel over batch B=8 across 8 NeuronCores. Per core:
  - emb[m, :] = mean(bert[s_m : e_m+1, :]) computed as a banded matmul
    A_tile.T @ bert_rows on the PE (float32r). A is a 0/1 span mask built on
    the host from mention_bounds; the 1/len scaling is applied during PSUM
    eviction as a per-partition scalar multiply (DVE/ACT alternating).
  - gold id matching via f32 compare/select ops on the gpsimd engine.
scores/bounds outputs are identity reshapes of the inputs (done host-side).
"""

import os
import numpy as np

import concourse.bass as bass
import concourse.tile as tile
from concourse import bacc, mybir
from concourse.bass_utils import run_bass_kernel_spmd

B, S, H, M, G = 8, 512, 768, 5075, 3
NCORES = 8
NT = (M + 127) // 128          # 40 span tiles of 128
MP = NT * 128                  # 5120 padded span count
NF = 40                        # free-dim size for the f-major [128, NF] gold layout
# bert is staged in SBUF as 8 overlapping row-chunks: chunk ci holds rows
# [64ci, min(64ci+128, S)). Piece starts are aligned to the 64-grid so every
# matmul operand starts at partition 0 (PE base-partition constraint), and
# the K extent stays <= ~85, avoiding a PE pipeline stall seen at K=128.
NCHUNK = 8

LAST_EXEC_NS = None            # set when KERNEL_TRACE=1

_prog_cache = {}


def _plan_pieces(smin, emax):
    """Per span-tile, decompose the token window [smin, emax] into pieces that
    each live inside one SBUF bert chunk, starting at partition 0.
    Returns list (per tile) of lists of (ci, nrows, kbase)."""
    plans = []
    for t in range(NT):
        lo, hi = int(smin[t]), int(emax[t])
        pieces = []
        pos = (lo // 64) * 64
        while pos <= hi:
            ci = pos // 64
            room = 128 if ci < NCHUNK - 1 else S - 64 * ci
            n = min(hi + 1 - pos, room)
            pieces.append((ci, n, pos))
            pos += n
        plans.append(pieces)
    return plans


def _build_program(plans):
    """Build + compile the SPMD Bass program for a given piece plan."""
    PT = sum(len(p) for p in plans)
    nc = bacc.Bacc("TRN2", target_bir_lowering=False, debug=False)
    f32, i32 = mybir.dt.float32, mybir.dt.int32
    f32r, bf16 = mybir.dt.float32r, mybir.dt.bfloat16

    # piece index ranges per A group: a small first group (so tile 0's
    # stationary operand lands fast) then ~12-piece groups.
    abounds = [0, min(4, PT)]
    while abounds[-1] < PT:
        abounds.append(min(abounds[-1] + 12, PT))
    nag = len(abounds) - 1

    bert_in = nc.dram_tensor("bert", [S, H], f32r, kind="ExternalInput")
    a_in = nc.dram_tensor("amat", [128, PT * 128], f32r, kind="ExternalInput")
    inv_in = nc.dram_tensor("inv", [128, NT], f32, kind="ExternalInput")
    se_in = nc.dram_tensor("se", [128, 2 * NF], f32, kind="ExternalInput")
    gold_in = nc.dram_tensor("gold", [128, 3 * G], f32, kind="ExternalInput")
    emb_out = nc.dram_tensor("emb", [MP, H], f32, kind="ExternalOutput")
    gid_out = nc.dram_tensor("gid", [128, NF], i32, kind="ExternalOutput")

    with tile.TileContext(nc) as tc:
        with (
            tc.tile_pool(name="const", bufs=1) as const,
            tc.tile_pool(name="psum", bufs=4, space="PSUM") as psum,
            tc.tile_pool(name="outp", bufs=8) as outp,
            tc.tile_pool(name="gtmp", bufs=2) as gtmp,
        ):
            # A group 0 first so tile 0's matmuls can start immediately.
            a_sb = [None] * nag

            def load_a(g, eng):
                lo, hi = abounds[g], abounds[g + 1]
                at = const.tile([128, (hi - lo) * 128], f32r, tag=f"amat{g}")
                eng.dma_start(at[:], a_in[:, lo * 128:hi * 128])
                a_sb[g] = at

            # Queue plan: the three DMA queues (sync/scalar HWDGE, gpsimd
            # SWDGE) are load-balanced, and transfers are ordered by when
            # their consumer tiles run (per-DMA completion latency ~2us does
            # not pipeline well within a single queue).
            inv_sb = const.tile([128, NT], f32, tag="inv")
            nc.gpsimd.dma_start(inv_sb[:], inv_in[:])

            bert_sb = [None] * NCHUNK

            def load_chunk(ci):
                bt = const.tile([128, H], f32r, tag=f"bert{ci}")
                nc.sync.dma_start(bt[:], bert_in[64 * ci:64 * ci + 128, :])
                bert_sb[ci] = bt

            # Ordered by when the consumer tiles run: tile t uses chunk
            # ~t/5 and A group covering piece t.
            load_a(0, nc.scalar)   # scalar queue is idle until evictions
            load_chunk(0)
            if nag > 1:
                load_a(1, nc.sync)
            for ci in range(1, NCHUNK, 2):
                bt = const.tile([128, H], f32r, tag=f"bert{ci}")
                nr = min(128, S - 64 * ci)
                nc.scalar.dma_start(bt[0:nr, :], bert_in[64 * ci:64 * ci + nr, :])
                bert_sb[ci] = bt
            load_chunk(2)
            if nag > 2:
                load_a(2, nc.sync)
            load_chunk(4)
            for g in range(3, nag):
                load_a(g, nc.sync)
            load_chunk(6)

            se_sb = const.tile([128, 2 * NF], f32, tag="se")
            nc.gpsimd.dma_start(se_sb[:], se_in[:])
            gold_sb = const.tile([128, 3 * G], f32, tag="gold")
            nc.gpsimd.dma_start(gold_sb[:], gold_in[:])

            pidx = 0
            for t in range(NT):
                ot = outp.tile([128, H], f32, tag="ot")
                ps = psum.tile([128, H], f32, tag="ps")
                np_ = len(plans[t])
                for j, (ci, k, _) in enumerate(plans[t]):
                    g = next(i for i in range(nag)
                             if abounds[i] <= pidx < abounds[i + 1])
                    off = pidx - abounds[g]
                    lhsT = a_sb[g][0:k, off * 128:(off + 1) * 128]
                    rhs = bert_sb[ci]
                    first, last = (j == 0), (j == np_ - 1)
                    nc.tensor.matmul(ps[:, 0:512], lhsT, rhs[0:k, 0:512],
                                     start=first, stop=last)
                    nc.tensor.matmul(ps[:, 512:H], lhsT, rhs[0:k, 512:H],
                                     start=first, stop=last)
                    pidx += 1
                iv = inv_sb[:, t:t + 1]
                if t % 2 == 0:
                    nc.vector.tensor_scalar_mul(ot[:], ps[:], iv)
                else:
                    nc.scalar.mul(ot[:], ps[:], iv)
                eng = nc.sync if t % 2 == 0 else nc.scalar
                eng.dma_start(emb_out[128 * t:128 * (t + 1), :], ot[:])

            # gold matching in f32 (positions <=512, ids <1e5 are f32-exact):
            # acc = acc*(1-m) + id*m per gold; later golds overwrite earlier.
            starts = se_sb[:, 0:NF]
            ends = se_sb[:, NF:2 * NF]
            acc = gtmp.tile([128, NF], f32, tag="acc")
            nc.gpsimd.memset(acc[:], -1.0)
            for g in range(G):
                m1 = gtmp.tile([128, NF], f32, tag="m1")
                m2 = gtmp.tile([128, NF], f32, tag="m2")
                nc.gpsimd.tensor_scalar(m1[:], starts, gold_sb[:, g:g + 1],
                                        None, mybir.AluOpType.is_equal)
                nc.gpsimd.tensor_scalar(m2[:], ends, gold_sb[:, G + g:G + g + 1],
                                        None, mybir.AluOpType.is_equal)
                nc.gpsimd.tensor_tensor(m1[:], m1[:], m2[:],
                                        mybir.AluOpType.mult)
                # tmp = m * id ; nm = 1 - m ; acc = acc*nm + tmp
                x1 = gtmp.tile([128, NF], f32, tag="x1")
                nc.gpsimd.tensor_scalar(x1[:], m1[:],
                                        gold_sb[:, 2 * G + g:2 * G + g + 1],
                                        None, mybir.AluOpType.mult)
                nc.gpsimd.tensor_scalar(m1[:], m1[:], -1.0, 1.0,
                                        mybir.AluOpType.mult,
                                        mybir.AluOpType.add)
                nc.gpsimd.tensor_tensor(acc[:], acc[:], m1[:],
                                        mybir.AluOpType.mult)
                nc.gpsimd.tensor_tensor(acc[:], acc[:], x1[:],
                                        mybir.AluOpType.add)
            acc_i = gtmp.tile([128, NF], i32, tag="acci")
            nc.gpsimd.tensor_copy(acc_i[:], acc[:])
            nc.sync.dma_start(gid_out[:], acc_i[:])

    nc.compile()
    return nc


def _host_prep(bert_output, mention_bounds, gold_mention_bounds,
               gold_mention_bounds_mask, gold_entity_local_id):
    starts = mention_bounds[..., 0].astype(np.int64)   # [B, M]
    ends = mention_bounds[..., 1].astype(np.int64)     # [B, M] inclusive

    # union window across cores per span tile (keeps the program SPMD-uniform)
    s_pad = np.full((B, MP), 0, np.int64)
    e_pad = np.full((B, MP), -1, np.int64)
    s_pad[:, :M] = starts
    e_pad[:, :M] = ends
    st = s_pad.reshape(B, NT, 128)
    et = e_pad.reshape(B, NT, 128)
    # padded columns have s=0,e=-1 (empty span); ignore them for the window
    smin = np.where(et >= st, st, np.iinfo(np.int64).max).min(axis=(0, 2))
    emax = np.where(et >= st, et, -1).max(axis=(0, 2))
    smin = np.minimum(smin, emax)          # guard (never hit with real spans)
    smin = np.clip(smin, 0, S - 1)
    emax = np.clip(emax, 0, S - 1)
    plans = _plan_pieces(smin, emax)
    PT = sum(len(p) for p in plans)

    inv_len = (np.float32(1.0) / (ends - starts + 1).astype(np.float32))  # [B, M]
    inv_pad = np.zeros((B, MP), np.float32)
    inv_pad[:, :M] = inv_len
    inv = np.ascontiguousarray(inv_pad.reshape(B, NT, 128).transpose(0, 2, 1))

    # A mask matrices per core: [128 rows(k), PT, 128 cols(span j)] in bf16
    amats = np.zeros((B, 128, PT, 128), np.float16).astype(np.dtype("bfloat16")
        ) if False else np.zeros((B, 128, PT, 128), np.float32)
    pidx = 0
    for t in range(NT):
        s_t, e_t = st[:, t], et[:, t]                            # [B, 128]
        for (ci, k, kbase) in plans[t]:
            kk = np.arange(kbase, kbase + k)[None, :, None]      # [1, k, 1]
            msk = (kk >= s_t[:, None, :]) & (kk <= e_t[:, None, :])
            amats[:, :k, pidx, :] = msk
            pidx += 1
    amats = amats.reshape(B, 128, PT * 128)

    # starts/ends in f-major [128, NF] layout (m = p + 128 f), pad sentinel -7
    def fmaj(x, pad):
        xp = np.full((B, MP), pad, np.float32)
        xp[:, :M] = x
        return xp.reshape(B, NF, 128).transpose(0, 2, 1)  # [B, 128, NF]

    se = np.concatenate([fmaj(starts, -7), fmaj(ends, -7)], axis=2)  # [B,128,2NF]
    se = np.ascontiguousarray(se, np.float32)

    # gold triples (start, end_incl, id) with sentinels for masked/invalid
    gs = np.full((B, G), -99999, np.int64)
    ge = np.full((B, G), -99999, np.int64)
    gi = np.zeros((B, G), np.int64)
    gm = gold_mention_bounds_mask.astype(bool)
    g0 = gold_mention_bounds[..., 0].astype(np.int64)
    g1 = gold_mention_bounds[..., 1].astype(np.int64) - 1   # inclusive end
    valid = gm & (g0 >= 0)
    gs = np.where(valid, g0, gs)
    ge = np.where(valid, g1, ge)
    gi = np.where(valid, gold_entity_local_id.astype(np.int64), gi)
    gold = np.concatenate([gs, ge, gi], axis=1).astype(np.float32)  # [B, 3G]
    gold = np.ascontiguousarray(np.broadcast_to(gold[:, None, :], (B, 128, 3 * G)))

    plan_key = tuple(tuple(p) for p in plans)
    return plans, plan_key, amats, inv, se, gold


def kernel(bert_output, mention_scores, mention_bounds, gold_mention_bounds,
           gold_mention_bounds_mask, gold_entity_local_id):
    global LAST_EXEC_NS
    assert bert_output.shape == (B, S, H)
    assert mention_bounds.shape == (B, M, 2)

    plans, plan_key, amats, inv, se, gold = _host_prep(
        bert_output, mention_bounds, gold_mention_bounds,
        gold_mention_bounds_mask, gold_entity_local_id)

    if plan_key not in _prog_cache:
        _prog_cache[plan_key] = _build_program(plans)
    nc = _prog_cache[plan_key]

    bert = np.ascontiguousarray(bert_output, dtype=np.float32)
    in_maps = [
        {"bert": bert[c], "amat": amats[c], "inv": inv[c],
         "se": se[c], "gold": gold[c]}
        for c in range(NCORES)
    ]

    trace = os.environ.get("KERNEL_TRACE", "") == "1"
    if trace:
        try:
            import sys, types
            from trn_agent_boot.trn_boot import _ntff_profile_via_ctypes
            hook = _ntff_profile_via_ctypes("/opt/axon/libaxon_pjrt.so")
            mod = types.ModuleType("antenv.axon_hooks")
            mod.get_axon_ntff_profile_hook = lambda: hook
            sys.modules["antenv.axon_hooks"] = mod
        except Exception:
            trace = False

    res = run_bass_kernel_spmd(nc, in_maps, list(range(NCORES)), trace=trace)
    LAST_EXEC_NS = res.exec_time_ns

    embs = np.concatenate([res.results[c]["emb"][:M] for c in range(NCORES)], 0)
    gids = np.concatenate(
        [res.results[c]["gid"].T.ravel()[:M] for c in range(NCORES)], 0)

    return (
        embs.astype(np.float32, copy=False),
        np.asarray(mention_scores, np.float32).reshape(-1, 1),
        np.asarray(mention_bounds, np.int32).reshape(-1, 2),
        gids.astype(np.int32, copy=False).reshape(-1, 1),
    )
